# revision 1
# baseline (speedup 1.0000x reference)
"""DeepseekV2 MLA attention kernel for Trainium2, 8-core tensor-parallel.

Strategy (per sharding hint): shard heads across the 8 cores (16 heads
each). q_b / kc / vc / o_w are sliced per-head on host; q_a and kv_a are
computed replicated on every core. Each core produces a partial [S, D]
output (its heads' contribution through o_proj); host sums the partials.

All device matmuls contract over the SBUF partition dimension, so the
whole computation is laid out "transposed" ([feature, seq]):
  hT [D, S]  ->  aT = qa_w^T @ hT  [1536, S]   (rmsnorm via ones-matmul
  partition sums of squares, rsqrt row broadcast by K=1 ones matmul)
  qT = qb^T @ aT_norm per head-pair, with per-pair column layout
  [nope(h0) nope(h1) pe(h0)|pe(h1) pe_rot(h0)|pe_rot(h1)], where pe_rot
  columns are pre-rotated/negated copies of pe columns so that RoPE is
  just  rope = pe*cosT + pe_rot*sinT  with no cross-partition moves.
  kvT likewise from kv_a_w extended with duplicated pe / pe_rot columns.
  scores^T[k,q] per head accumulate 5 matmuls (4x latent d-chunks + pe),
  exp on ACT (scale folded in; no max-subtraction needed -- logits are
  O(5)), causal mask via 0/1 mask multiply on the 8 diagonal tiles,
  softmax denominator via ones-column matmul, recip broadcast applied on
  the attn psum. out_v^T = vc^T @ attn^T; o_proj accumulates over the
  2048 local head*v dims into [S, D] partials.
"""
import sys
import math

sys.path.insert(0, '/opt/trn_rl_repo')

import numpy as np
import ml_dtypes
from contextlib import ExitStack

import concourse.bass as bass
import concourse.tile as tile
from concourse import bacc, mybir
from concourse.masks import make_identity

# ---- problem constants (hardcoded; kernel.py must be self-contained) ----
H = 128
D = 5120
Q_LORA = 1536
KV_LORA = 512
ROPE = 64
NOPE = 128
VDIM = 128
Q_HEAD = NOPE + ROPE
S = 1024
EPS = 1e-6
_MSCALE = 0.1 * 1.0 * math.log(40.0) + 1.0
SCALE = (Q_HEAD ** -0.5) * _MSCALE * _MSCALE

NCORES = 8
HLOC = H // NCORES          # 16 heads per core
QB_COLS = HLOC * (NOPE + 2 * ROPE)   # 4096
KV_COLS = KV_LORA + 4 * ROPE         # 512 latent + pe,pe,rot,rot = 768
OW_ROWS = HLOC * VDIM                # 2048

F32 = mybir.dt.float32
BF16 = mybir.dt.bfloat16

DK = D // 128          # 40 d-chunks
QK = Q_LORA // 128     # 12 q_lora chunks
SQ = S // 512          # 2 free-dim chunks of 512
SK = S // 128          # 8 key chunks of 128
LC = KV_LORA // 128    # 4 latent chunks
NPAIR = HLOC // 2      # 8 head pairs

bf16 = ml_dtypes.bfloat16


def build_program(reps=1, upto=3):
    nc = bacc.Bacc("TRN2", target_bir_lowering=False, debug=False,
                   num_devices=NCORES)

    hT_d = nc.dram_tensor("hT", [D, S], BF16, kind="ExternalInput").ap()
    cos2_d = nc.dram_tensor("cos2T", [128, S], BF16, kind="ExternalInput").ap()
    sin2_d = nc.dram_tensor("sin2T", [128, S], BF16, kind="ExternalInput").ap()
    qa_d = nc.dram_tensor("qa_w", [QK, 128, D], BF16, kind="ExternalInput").ap()
    qb_d = nc.dram_tensor("qb_w", [QB_COLS // 128, 128, Q_LORA], BF16,
                          kind="ExternalInput").ap()
    kvw_d = nc.dram_tensor("kv_w", [KV_COLS // 128, 128, D], BF16,
                           kind="ExternalInput").ap()
    kc_d = nc.dram_tensor("kc_w", [HLOC, NOPE, KV_LORA], BF16, kind="ExternalInput").ap()
    vc_d = nc.dram_tensor("vc_w", [HLOC, KV_LORA, VDIM], BF16, kind="ExternalInput").ap()
    ow_d = nc.dram_tensor("o_w", [OW_ROWS, D], BF16, kind="ExternalInput").ap()
    mask_d = nc.dram_tensor("masks", [4, 128, 512], BF16, kind="ExternalInput").ap()
    out_d = nc.dram_tensor("out", [S, D], F32, kind="ExternalOutput").ap()

    with tile.TileContext(nc) as tc, \
         nc.allow_low_precision(reason="float32r rows are fp32-width"):
      for _rep in range(reps):
       with ExitStack() as ctx:
        const = ctx.enter_context(tc.tile_pool(name="const", bufs=1))
        persist = ctx.enter_context(tc.tile_pool(name="persist", bufs=1))

        # ---- constants ----
        ident = const.tile([128, 128], BF16)
        make_identity(nc, ident)
        ones_col = const.tile([128, 1], BF16)   # lhsT for partition sums
        nc.vector.memset(ones_col, 1.0)
        eps_sb = const.tile([1, 1], F32)
        nc.vector.memset(eps_sb, EPS)
        cos2 = const.tile([128, S], BF16)
        nc.sync.dma_start(cos2, cos2_d)
        sin2 = const.tile([128, S], BF16)
        nc.sync.dma_start(sin2, sin2_d)
        masks = []
        for i in range(4):
            m_t = const.tile([128, 512], BF16, name=f"mask{i}")
            nc.sync.dma_start(m_t, mask_d[i])
            masks.append(m_t)

        # persistent activations (whole-program scope)
        a_sb = [persist.tile([128, S], BF16, name=f"a{m}") for m in range(QK)]
        lat_T = [persist.tile([128, S], BF16, name=f"latT{m}") for m in range(LC)]
        lat_kl = [persist.tile([128, KV_LORA], BF16, name=f"latkl{k}")
                  for k in range(SK)]
        kv_pe = persist.tile([128, S], BF16, name="kv_pe")

        # =========== phase 1: aT = qa^T @ hT, kvT = kvw^T @ hT ===========
        with tc.tile_pool(name="p1", bufs=2) as p1, \
             tc.tile_pool(name="p1s", bufs=4) as p1s:
            hT = []
            for k in range(DK):
                h_t = p1.tile([128, S], BF16, name=f"hT{k}", bufs=1)
                nc.sync.dma_start(h_t, hT_d[k * 128:(k + 1) * 128, :])
                hT.append(h_t)


            ss_a = p1s.tile([1, S], F32, bufs=1)
            nc.vector.memset(ss_a, 0.0)
            ss_kv = p1s.tile([1, S], F32, bufs=1)
            nc.vector.memset(ss_kv, 0.0)

            kv_lat_raw = [p1s.tile([128, S], BF16, name=f"kvraw{m}", bufs=1)
                          for m in range(LC)]
            kv_rot = p1s.tile([128, S], BF16, name="kv_rot", bufs=1)

            with tc.tile_pool(name="psA", bufs=1, space="PSUM") as psA:
                for m in range(QK + KV_COLS // 128):
                    is_q = m < QK
                    mm = m if is_q else m - QK
                    w_t = p1.tile([128, DK, 128], BF16, name="w_t",
                                  tag="w_stream")
                    src = qa_d if is_q else kvw_d
                    nc.sync.dma_start(
                        w_t, src[mm].rearrange("p (k c) -> p k c", c=128))
                    for qc in range(SQ):
                        acc = psA.tile([128, 512], F32, name="acc",
                                       tag="p1acc", bufs=3)
                        for k in range(DK):
                            nc.tensor.matmul(acc, w_t[:, k, :],
                                             hT[k][:, qc * 512:(qc + 1) * 512],
                                             start=(k == 0), stop=(k == DK - 1))
                        if is_q:
                            dst = a_sb[mm]
                        elif mm < LC:
                            dst = kv_lat_raw[mm]
                        elif mm == LC:
                            dst = kv_pe
                        else:
                            dst = kv_rot
                        nc.vector.tensor_copy(dst[:, qc * 512:(qc + 1) * 512], acc)
                        if is_q or mm < LC:
                            sq = p1.tile([128, 512], BF16, name="sq", tag="sq")
                            nc.scalar.square(sq, acc)
                            sqs = psA.tile([1, 512], F32, name="sqs",
                                           tag="sqs", bufs=2)
                            nc.tensor.matmul(sqs, ones_col, sq,
                                             start=True, stop=True)
                            tgt = ss_a if is_q else ss_kv
                            nc.vector.tensor_add(
                                tgt[:, qc * 512:(qc + 1) * 512],
                                tgt[:, qc * 512:(qc + 1) * 512], sqs)

            # rsqrt rows
            rstd_a = p1s.tile([1, S], F32, bufs=1)
            nc.scalar.activation(rstd_a, ss_a, mybir.ActivationFunctionType.Sqrt,
                                 bias=eps_sb, scale=1.0 / Q_LORA)
            nc.vector.reciprocal(rstd_a, rstd_a)
            rstd_kv = p1s.tile([1, S], F32, bufs=1)
            nc.scalar.activation(rstd_kv, ss_kv, mybir.ActivationFunctionType.Sqrt,
                                 bias=eps_sb, scale=1.0 / KV_LORA)
            nc.vector.reciprocal(rstd_kv, rstd_kv)

            for qc in range(SQ):
                sl = slice(qc * 512, (qc + 1) * 512)
                bc_a = p1s.tile([128, 512], F32, name="bc_a", tag="bc", bufs=2)
                nc.gpsimd.partition_broadcast(bc_a, rstd_a[:, sl])
                for m in range(QK):
                    nc.vector.tensor_mul(a_sb[m][:, sl], a_sb[m][:, sl], bc_a)
                bc_kv = p1s.tile([128, 512], F32, name="bc_kv", tag="bc", bufs=2)
                nc.gpsimd.partition_broadcast(bc_kv, rstd_kv[:, sl])
                for m in range(LC):
                    nc.vector.tensor_mul(lat_T[m][:, sl],
                                         kv_lat_raw[m][:, sl], bc_kv)

            # k_pe rope (pe duplicated in both partition halves by construction)
            nc.vector.tensor_mul(kv_pe, kv_pe, cos2)
            nc.vector.tensor_mul(kv_rot, kv_rot, sin2)
            nc.vector.tensor_add(kv_pe, kv_pe, kv_rot)

            # latent transpose -> [k, l] tiles
            with tc.tile_pool(name="psC", bufs=1, space="PSUM") as psC:
                for k in range(SK):
                    for lc in range(LC):
                        tp = psC.tile([128, 128], BF16, name="tp", tag="tp", bufs=4)
                        nc.tensor.transpose(
                            tp, lat_T[lc][:, k * 128:(k + 1) * 128], ident)
                        nc.vector.tensor_copy(
                            lat_kl[k][:, lc * 128:(lc + 1) * 128], tp)

        if upto < 2:
            with tc.tile_pool(name="anchor", bufs=1) as ap_, \
                 tc.tile_pool(name="psAn", bufs=1, space="PSUM") as psan:
                ob = ap_.tile([128, 512], F32, name="ob_anchor")
                nc.vector.tensor_copy(ob[:, 0:KV_LORA], lat_kl[0])
                nc.vector.tensor_copy(ob[:, 0:128], kv_pe[:, 0:128])
                nc.vector.tensor_copy(ob[:, 128:256], a_sb[QK - 1][:, 0:128])
                nc.sync.dma_start(out_d[0:128, 0:512], ob)
            continue

        # ====== phase 2+3: per head pair: q_b, rope, attention ======
        with tc.tile_pool(name="ov", bufs=1) as ovp:
            out_v = [ovp.tile([128, S], BF16, name=f"ov{h}") for h in range(HLOC)]

            with tc.tile_pool(name="ph", bufs=4) as ph, \
                 tc.tile_pool(name="pp", bufs=8) as pp, \
                 tc.tile_pool(name="psH", bufs=1, space="PSUM") as psH:
                for pr in range(NPAIR):
                    # --- q_b for this pair: 4 column chunks of 128 ---
                    q_nope = [ph.tile([128, S], BF16, name=f"qn{e}",
                                      tag=f"qn{e}", bufs=2) for e in (0, 1)]
                    q_pe = ph.tile([128, S], BF16, name="qpe", tag="qpe", bufs=2)
                    q_rot = ph.tile([128, S], BF16, name="qrot", tag="qrot", bufs=2)
                    dsts = [q_nope[0], q_nope[1], q_pe, q_rot]
                    for cc in range(4):
                        w_t = ph.tile([128, QK, 128], BF16, name="qb_t",
                                      tag="qb_stream", bufs=3)
                        nc.sync.dma_start(
                            w_t,
                            qb_d[pr * 4 + cc].rearrange("p (k c) -> p k c", c=128))
                        for qc in range(SQ):
                            sl = slice(qc * 512, (qc + 1) * 512)
                            acc = psH.tile([128, 512], F32, name="acc2",
                                           tag="hmm", bufs=2)
                            for k in range(QK):
                                nc.tensor.matmul(acc, w_t[:, k, :],
                                                 a_sb[k][:, sl],
                                                 start=(k == 0),
                                                 stop=(k == QK - 1))
                            nc.vector.tensor_copy(dsts[cc][:, sl], acc)
                    # rope: q_pe = q_pe*cos + q_rot*sin  (both heads at once)
                    nc.vector.tensor_mul(q_pe, q_pe, cos2)
                    nc.vector.tensor_mul(q_rot, q_rot, sin2)
                    nc.vector.tensor_add(q_pe, q_pe, q_rot)

                    kc_sbs, vc_sbs, qabs = [], [], []
                    for e in (0, 1):
                        h = 2 * pr + e
                        kc_sb = ph.tile([128, KV_LORA], BF16, name=f"kc{e}",
                                        tag=f"kc_w{e}")
                        nc.sync.dma_start(kc_sb, kc_d[h])
                        vc_sb = ph.tile([128, LC, VDIM], BF16, name=f"vc{e}",
                                        tag=f"vc_w{e}")
                        nc.sync.dma_start(
                            vc_sb, vc_d[h].rearrange("(lc p) v -> p lc v", p=128))
                        kc_sbs.append(kc_sb)
                        vc_sbs.append(vc_sb)
                        qabs.append([ph.tile([128, S], BF16, name=f"qabs{e}{lc}",
                                             tag=f"qabs{e}{lc}", bufs=2)
                                     for lc in range(LC)])

                    for e in (0, 1):
                        for lc in range(LC):
                            for qc in range(SQ):
                                sl = slice(qc * 512, (qc + 1) * 512)
                                acc = psH.tile([128, 512], F32, name="acckc",
                                               tag="hmm", bufs=2)
                                nc.tensor.matmul(
                                    acc, kc_sbs[e][:, lc * 128:(lc + 1) * 128],
                                    q_nope[e][:, sl], start=True, stop=True)
                                nc.vector.tensor_copy(qabs[e][lc][:, sl], acc)

                    for qc in range(SQ):
                        sl = slice(qc * 512, (qc + 1) * 512)
                        nkc = 4 if qc == 0 else SK
                        probs_all = {0: [], 1: []}
                        ssums = {}
                        atts = {}
                        for e in (0, 1):
                            ssums[e] = psH.tile([1, 512], F32, name=f"ssum{e}",
                                                tag=f"ssum{e}", bufs=1)
                            atts[e] = [psH.tile([128, 512], F32,
                                                name=f"att{e}{half}",
                                                tag=f"att{e}{half}", bufs=1)
                                       for half in (0, 1)]
                        # pass 0: scores + exp + mask + ssum + att lc 0,1
                        for kc in range(nkc):
                            ks = slice(kc * 128, (kc + 1) * 128)
                            for e in (0, 1):
                                pe_b = e * 64
                                sc = psH.tile([128, 512], F32, name="sc",
                                              tag="hmm", bufs=2)
                                for lc in range(LC):
                                    nc.tensor.matmul(sc, lat_T[lc][:, ks],
                                                     qabs[e][lc][:, sl],
                                                     start=(lc == 0), stop=False)
                                nc.tensor.matmul(
                                    sc, kv_pe[pe_b:pe_b + 64, ks],
                                    q_pe[pe_b:pe_b + 64, sl],
                                    start=False, stop=True)
                                probs = pp.tile([128, 512], BF16,
                                                name=f"probs{e}",
                                                tag=f"probs{e}", bufs=10)
                                nc.scalar.activation(
                                    probs, sc, mybir.ActivationFunctionType.Exp,
                                    scale=SCALE)
                                midx = kc - (0 if qc == 0 else 4)
                                if midx >= 0:
                                    nc.vector.tensor_mul(probs, probs, masks[midx])
                                probs_all[e].append(probs)
                                nc.tensor.matmul(ssums[e], ones_col, probs,
                                                 start=(kc == 0),
                                                 stop=(kc == nkc - 1))
                                for lc in (0, 1):
                                    nc.tensor.matmul(
                                        atts[e][lc],
                                        lat_kl[kc][:, lc * 128:(lc + 1) * 128],
                                        probs, start=(kc == 0),
                                        stop=(kc == nkc - 1))
                        # drain pass-0 att banks, then pass 1 reuses them
                        asb = {0: {}, 1: {}}
                        for e in (0, 1):
                            for lc in (0, 1):
                                t = pp.tile([128, 512], BF16, name=f"asb{e}",
                                            tag=f"asb{e}", bufs=5)
                                nc.vector.tensor_copy(t, atts[e][lc])
                                asb[e][lc] = t
                        for e in (0, 1):
                            atts[e] = [psH.tile([128, 512], F32,
                                                name=f"att{e}{half}b",
                                                tag=f"att{e}{half}", bufs=1)
                                       for half in (0, 1)]
                        for kc in range(nkc):
                            for e in (0, 1):
                                for i, lc in enumerate((2, 3)):
                                    nc.tensor.matmul(
                                        atts[e][i],
                                        lat_kl[kc][:, lc * 128:(lc + 1) * 128],
                                        probs_all[e][kc], start=(kc == 0),
                                        stop=(kc == nkc - 1))
                        for e in (0, 1):
                            h = 2 * pr + e
                            recip = ph.tile([1, 512], F32, name=f"recip{e}",
                                            tag=f"recip{e}", bufs=2)
                            nc.vector.reciprocal(recip, ssums[e])
                            bc_sb = pp.tile([128, 512], F32, name=f"bc_sb{e}",
                                            tag=f"bcsb{e}", bufs=2)
                            nc.gpsimd.partition_broadcast(bc_sb, recip)
                            for i, lc in enumerate((2, 3)):
                                t = pp.tile([128, 512], BF16, name=f"asb{e}b",
                                            tag=f"asb{e}", bufs=5)
                                nc.vector.tensor_copy(t, atts[e][i])
                                asb[e][lc] = t
                            vout = psH.tile([128, 512], F32, name=f"vout{e}",
                                            tag="hmm", bufs=2)
                            for lc in range(LC):
                                nc.tensor.matmul(vout, vc_sbs[e][:, lc, :],
                                                 asb[e][lc],
                                                 start=(lc == 0),
                                                 stop=(lc == LC - 1))
                            nc.vector.tensor_mul(out_v[h][:, sl], vout, bc_sb)

            # =========== phase 6: o_proj partials ===========
            with tc.tile_pool(name="p6", bufs=6) as p6, \
                 tc.tile_pool(name="ps6", bufs=1, space="PSUM") as ps6:
                for dc in range(D // 512):
                    po = [ps6.tile([128, 512], F32, name=f"po{sc2}",
                                   tag=f"po{sc2}", bufs=1)
                          for sc2 in range(SK)]
                    for hv in range(HLOC):
                        ow_t = p6.tile([128, 512], BF16, name="ow_t",
                                       tag="ow_stream")
                        nc.sync.dma_start(
                            ow_t,
                            ow_d[hv * 128:(hv + 1) * 128, dc * 512:(dc + 1) * 512])
                        for sc2 in range(SK):
                            nc.tensor.matmul(
                                po[sc2], out_v[hv][:, sc2 * 128:(sc2 + 1) * 128],
                                ow_t, start=(hv == 0), stop=(hv == HLOC - 1))
                    for sc2 in range(SK):
                        ob = p6.tile([128, 512], F32, name="ob", tag="ob")
                        nc.scalar.copy(ob, po[sc2])
                        nc.sync.dma_start(
                            out_d[sc2 * 128:(sc2 + 1) * 128,
                                  dc * 512:(dc + 1) * 512], ob)

    nc.compile()
    return nc


def host_prep(hidden_states, cos, sin, q_a_w, q_a_ln_w, q_b_w, kv_a_w,
              kv_a_ln_w, kc_w, vc_w, o_w):
    """Shard + preprocess full fp32 inputs into per-core bf16 in_maps."""
    h = np.asarray(hidden_states)[0]            # [S, D]
    hT = np.ascontiguousarray(h.T).astype(bf16)  # [D, S]
    cosT = np.ascontiguousarray(np.asarray(cos).T)  # [64, S]
    sinT = np.ascontiguousarray(np.asarray(sin).T)
    cos2T = np.concatenate([cosT, cosT], 0).astype(bf16)   # [128, S]
    sin2T = np.concatenate([sinT, sinT], 0).astype(bf16)

    qa0 = np.asarray(q_a_w).astype(bf16)         # [D, 1536]
    qa = np.ascontiguousarray(
        qa0.reshape(DK, 128, QK, 128).transpose(2, 1, 0, 3).reshape(QK, 128, D))
    qb = (np.asarray(q_b_w) * np.asarray(q_a_ln_w)[:, None])  # ln folded

    lat_w = np.asarray(kv_a_w)[:, :KV_LORA]
    pe_w = np.asarray(kv_a_w)[:, KV_LORA:]
    rot_w = np.concatenate([-pe_w[:, ROPE // 2:], pe_w[:, :ROPE // 2]], 1)
    kvw0 = np.concatenate([lat_w, pe_w, pe_w, rot_w, rot_w], 1).astype(bf16)
    nkv = KV_COLS // 128
    kvw = np.ascontiguousarray(
        kvw0.reshape(DK, 128, nkv, 128).transpose(2, 1, 0, 3).reshape(nkv, 128, D))

    kv_ln = np.asarray(kv_a_ln_w)
    kc_eff = (np.asarray(kc_w) * kv_ln[None, None, :]).astype(bf16)  # [H,128,512]
    vc_eff = (np.asarray(vc_w) * kv_ln[None, :, None]).astype(bf16)  # [H,512,128]
    ow = np.asarray(o_w)

    i = np.arange(128)[:, None]
    j = np.arange(512)[None, :]
    masks = np.stack([(j >= i + off).astype(np.float32)
                      for off in (0, 128, 256, 384)]).astype(bf16)

    in_maps = []
    for c in range(NCORES):
        blocks = []
        for p in range(NPAIR):
            h0 = c * HLOC + 2 * p
            h1 = h0 + 1
            n0 = qb[:, h0 * Q_HEAD: h0 * Q_HEAD + NOPE]
            n1 = qb[:, h1 * Q_HEAD: h1 * Q_HEAD + NOPE]
            p0 = qb[:, h0 * Q_HEAD + NOPE: (h0 + 1) * Q_HEAD]
            p1 = qb[:, h1 * Q_HEAD + NOPE: (h1 + 1) * Q_HEAD]
            r0 = np.concatenate([-p0[:, ROPE // 2:], p0[:, :ROPE // 2]], 1)
            r1 = np.concatenate([-p1[:, ROPE // 2:], p1[:, :ROPE // 2]], 1)
            blocks += [n0, n1, np.concatenate([p0, p1], 1),
                       np.concatenate([r0, r1], 1)]
        qb_core0 = np.concatenate(blocks, axis=1).astype(bf16)  # [1536, 4096]
        nqb = QB_COLS // 128
        qb_core = np.ascontiguousarray(
            qb_core0.reshape(QK, 128, nqb, 128).transpose(2, 1, 0, 3)
            .reshape(nqb, 128, Q_LORA))
        in_maps.append({
            "hT": hT,
            "cos2T": cos2T,
            "sin2T": sin2T,
            "qa_w": qa,
            "qb_w": qb_core,
            "kv_w": kvw,
            "kc_w": kc_eff[c * HLOC:(c + 1) * HLOC],
            "vc_w": vc_eff[c * HLOC:(c + 1) * HLOC],
            "o_w": ow[c * OW_ROWS:(c + 1) * OW_ROWS].astype(bf16),
            "masks": masks,
        })
    return in_maps


_CACHE = {}


def _get_program():
    if "nc" not in _CACHE:
        _CACHE["nc"] = build_program()
    return _CACHE["nc"]


def kernel(**inputs) -> np.ndarray:
    from concourse.bass_utils import run_bass_kernel_spmd
    nc = _get_program()
    in_maps = host_prep(**inputs)
    res = run_bass_kernel_spmd(nc, in_maps, list(range(NCORES)))
    out = np.zeros((S, D), np.float64)
    for c in range(NCORES):
        out += res.results[c]["out"].astype(np.float64)
    return out.astype(np.float32)[None]



# revision 4
# speedup vs baseline: 1.0921x; 1.0921x over previous
"""DeepseekV2 MLA attention kernel for Trainium2, 8-core tensor-parallel.

Strategy: heads sharded across 8 cores (16 each) for attention + o_proj;
the shared q_a / kv_a projections are sharded by output chunk across
cores (2 chunks of 128 each) with an AllReduce for the RMS sums and an
AllGather for the normalized activations (vs. fully replicated in the
baseline -- saves ~250us of tensor-engine time per core).

All device matmuls contract over the SBUF partition dimension
("transposed" [feature, seq] layout):
  phase 1 (sharded): each core computes 2 of 16 global chunks of
    [aT(12); latent(4)] = W_chunk^T @ hT, plus the pe/pe and rot/rot
    chunks replicated (needed for k_pe RoPE with no partition moves).
    Sum-of-squares rows are routed by a per-core selector matmul
    (sel2: [128,2], col r = 1/N for the core's row type), AllReduced,
    rstd broadcast back to 128 partitions via a per-core [2,128]
    selector matmul, chunks normalized locally, then AllGathered.
  phase 2: qT = qb^T @ aT per head pair with pre-rotated pe columns so
    RoPE is elementwise; kc-absorbed q (qabs) per head.
  phase 3: causal attention at 256-wide query granularity (25% less
    masked-tile waste than 512): per (pair, head, 256-q-chunk), scores
    accumulate 5 matmuls per 128-key tile, exp on ACT, diagonal masks,
    softmax denominator via ones-column matmul, single-pass attn
    accumulation (4 latent chunks) in PSUM, vc bmm, recip broadcast.
  phase 6: o_proj accumulated over the 2048 local head*v dims into
    [S, D] partials; host sums the 8 partials.
"""
import sys
import math

sys.path.insert(0, '/opt/trn_rl_repo')

import numpy as np
import ml_dtypes
from contextlib import ExitStack

import concourse.bass as bass
import concourse.tile as tile
from concourse import bacc, mybir
from concourse.masks import make_identity

# ---- problem constants (hardcoded; kernel.py must be self-contained) ----
H = 128
D = 5120
Q_LORA = 1536
KV_LORA = 512
ROPE = 64
NOPE = 128
VDIM = 128
Q_HEAD = NOPE + ROPE
S = 1024
EPS = 1e-6
_MSCALE = 0.1 * 1.0 * math.log(40.0) + 1.0
SCALE = (Q_HEAD ** -0.5) * _MSCALE * _MSCALE

NCORES = 8
HLOC = H // NCORES          # 16 heads per core
QB_COLS = HLOC * (NOPE + 2 * ROPE)   # 4096
OW_ROWS = HLOC * VDIM                # 2048

F32 = mybir.dt.float32
BF16 = mybir.dt.bfloat16

DK = D // 128          # 40 d-chunks
QK = Q_LORA // 128     # 12 q_lora chunks
SQ = S // 512          # 2 free-dim chunks of 512
QV = S // 256          # 4 query chunks of 256 (attention granularity)
SK = S // 128          # 8 key chunks of 128
LC = KV_LORA // 128    # 4 latent chunks
NPAIR = HLOC // 2      # 8 head pairs
NCH = QK + LC          # 16 allgathered chunks
CPC = NCH // NCORES    # 2 chunks per core

bf16 = ml_dtypes.bfloat16


def build_program(reps=1, upto=3):
    nc = bacc.Bacc("TRN2", target_bir_lowering=False, debug=False,
                   num_devices=NCORES)

    hT_d = nc.dram_tensor("hT", [D, S], BF16, kind="ExternalInput").ap()
    cos2_d = nc.dram_tensor("cos2T", [128, S], BF16, kind="ExternalInput").ap()
    sin2_d = nc.dram_tensor("sin2T", [128, S], BF16, kind="ExternalInput").ap()
    ph1_d = nc.dram_tensor("ph1_w", [CPC, 128, D], BF16, kind="ExternalInput").ap()
    perot_d = nc.dram_tensor("perot_w", [2, 128, D], BF16,
                             kind="ExternalInput").ap()
    sel2_d = nc.dram_tensor("sel2", [128, 2], BF16, kind="ExternalInput").ap()
    selb_d = nc.dram_tensor("selb", [2, 128], F32, kind="ExternalInput").ap()
    qb_d = nc.dram_tensor("qb_w", [QB_COLS // 128, 128, Q_LORA], BF16,
                          kind="ExternalInput").ap()
    kc_d = nc.dram_tensor("kc_w", [HLOC, NOPE, KV_LORA], BF16, kind="ExternalInput").ap()
    vc_d = nc.dram_tensor("vc_w", [HLOC, KV_LORA, VDIM], BF16, kind="ExternalInput").ap()
    ow_d = nc.dram_tensor("o_w", [OW_ROWS, D], BF16, kind="ExternalInput").ap()
    mask_d = nc.dram_tensor("masks", [2, 128, 256], BF16, kind="ExternalInput").ap()
    out_d = nc.dram_tensor("out", [S, D], F32, kind="ExternalOutput").ap()
    rg = [list(range(NCORES))]

    with tile.TileContext(nc) as tc, \
         nc.allow_low_precision(reason="float32r rows are fp32-width"):
      for _rep in range(reps):
       with ExitStack() as ctx:
        const = ctx.enter_context(tc.tile_pool(name="const", bufs=1))
        persist = ctx.enter_context(tc.tile_pool(name="persist", bufs=1))
        dram = ctx.enter_context(tc.tile_pool(name="dram", bufs=1, space="DRAM"))

        # ---- constants ----
        ident = const.tile([128, 128], BF16)
        make_identity(nc, ident)
        ones_col = const.tile([128, 1], BF16)   # lhsT for partition sums
        nc.vector.memset(ones_col, 1.0)
        eps_sb2 = const.tile([2, 1], F32)
        nc.vector.memset(eps_sb2, EPS)
        sel2 = const.tile([128, 2], BF16)
        nc.sync.dma_start(sel2, sel2_d)
        selb = const.tile([2, 128], F32)
        nc.sync.dma_start(selb, selb_d)
        cos2 = const.tile([128, S], BF16)
        nc.sync.dma_start(cos2, cos2_d)
        sin2 = const.tile([128, S], BF16)
        nc.sync.dma_start(sin2, sin2_d)
        masks = []
        for i in range(2):
            m_t = const.tile([128, 256], BF16, name=f"mask{i}")
            nc.sync.dma_start(m_t, mask_d[i])
            masks.append(m_t)

        # persistent activations (whole-program scope)
        a_sb = [persist.tile([128, S], BF16, name=f"a{m}") for m in range(QK)]
        lat_T = [persist.tile([128, S], BF16, name=f"latT{m}") for m in range(LC)]
        lat_kl = [persist.tile([128, KV_LORA], BF16, name=f"latkl{k}")
                  for k in range(SK)]
        kv_pe = persist.tile([128, S], BF16, name="kv_pe")

        # collective bounce buffers
        ar_in = dram.tile([2, S], F32)
        ar_out = dram.tile([2, S], F32, addr_space="Shared")
        ag_in = dram.tile([CPC * 128, S], BF16)
        ag_out = dram.tile([NCH * 128, S], BF16, addr_space="Shared")

        # =========== phase 1 (sharded): own chunks + replicated pe/rot ====
        with tc.tile_pool(name="p1", bufs=2) as p1, \
             tc.tile_pool(name="p1s", bufs=1) as p1s:
            hT = []
            for k in range(DK):
                h_t = p1.tile([128, S], BF16, name=f"hT{k}", bufs=1)
                nc.sync.dma_start(h_t, hT_d[k * 128:(k + 1) * 128, :])
                hT.append(h_t)

            own = [p1s.tile([128, S], BF16, name=f"own{m}") for m in range(CPC)]
            kv_pe_raw = p1s.tile([128, S], BF16, name="kv_pe_raw")
            kv_rot = p1s.tile([128, S], BF16, name="kv_rot")
            ss_sb = p1s.tile([2, S], F32, name="ss_sb")

            with tc.tile_pool(name="psA", bufs=1, space="PSUM") as psA:
                ss2 = [psA.tile([2, 512], F32, name=f"ss2_{qc}", bufs=1)
                       for qc in range(SQ)]
                for m in range(CPC):
                    w_t = p1.tile([128, DK, 128], BF16, name="w_t",
                                  tag="w_stream")
                    nc.sync.dma_start(
                        w_t, ph1_d[m].rearrange("p (k c) -> p k c", c=128))
                    for qc in range(SQ):
                        sl = slice(qc * 512, (qc + 1) * 512)
                        acc = psA.tile([128, 512], F32, name="acc",
                                       tag="p1acc", bufs=3)
                        for k in range(DK):
                            nc.tensor.matmul(acc, w_t[:, k, :], hT[k][:, sl],
                                             start=(k == 0), stop=(k == DK - 1))
                        nc.vector.tensor_copy(own[m][:, sl], acc)
                        sq = p1.tile([128, 512], BF16, name="sq", tag="sq")
                        nc.scalar.square(sq, acc)
                        nc.tensor.matmul(ss2[qc], sel2, sq,
                                         start=(m == 0), stop=(m == CPC - 1))
                for qc in range(SQ):
                    sl = slice(qc * 512, (qc + 1) * 512)
                    nc.vector.tensor_copy(ss_sb[:, sl], ss2[qc])
                nc.sync.dma_start(ar_in[:], ss_sb)
                nc.gpsimd.collective_compute(
                    "AllReduce", mybir.AluOpType.add, replica_groups=rg,
                    ins=[ar_in[:]], outs=[ar_out[:]])

                # replicated pe/rot chunks (dup layout [pe;pe], [rot;rot])
                for m in range(2):
                    w_t = p1.tile([128, DK, 128], BF16, name="w_t2",
                                  tag="w_stream")
                    nc.sync.dma_start(
                        w_t, perot_d[m].rearrange("p (k c) -> p k c", c=128))
                    dst = kv_pe_raw if m == 0 else kv_rot
                    for qc in range(SQ):
                        sl = slice(qc * 512, (qc + 1) * 512)
                        acc = psA.tile([128, 512], F32, name="acc2",
                                       tag="p1acc", bufs=3)
                        for k in range(DK):
                            nc.tensor.matmul(acc, w_t[:, k, :], hT[k][:, sl],
                                             start=(k == 0), stop=(k == DK - 1))
                        nc.vector.tensor_copy(dst[:, sl], acc)

                # k_pe rope (pe duplicated in both partition halves)
                nc.vector.tensor_mul(kv_pe, kv_pe_raw, cos2)
                nc.vector.tensor_mul(kv_rot, kv_rot, sin2)
                nc.vector.tensor_add(kv_pe, kv_pe, kv_rot)

                # rstd from allreduced sums; normalize own chunks; allgather
                rstd_src = p1s.tile([2, S], F32, name="rstd_src")
                nc.sync.dma_start(rstd_src, ar_out[:])
                rstd2 = p1s.tile([2, S], F32, name="rstd2")
                nc.scalar.activation(rstd2, rstd_src,
                                     mybir.ActivationFunctionType.Sqrt,
                                     bias=eps_sb2, scale=1.0)
                nc.vector.reciprocal(rstd2, rstd2)
                for qc in range(SQ):
                    sl = slice(qc * 512, (qc + 1) * 512)
                    bc = psA.tile([128, 512], F32, name="bc", tag="bc", bufs=2)
                    nc.tensor.matmul(bc, selb, rstd2[:, sl],
                                     start=True, stop=True)
                    for m in range(CPC):
                        nc.vector.tensor_mul(own[m][:, sl], own[m][:, sl], bc)
                for m in range(CPC):
                    nc.sync.dma_start(ag_in[m * 128:(m + 1) * 128], own[m])
                nc.gpsimd.collective_compute(
                    "AllGather", mybir.AluOpType.bypass, replica_groups=rg,
                    ins=[ag_in[:]], outs=[ag_out[:]])

            # read back gathered activations
            for m in range(QK):
                nc.sync.dma_start(a_sb[m], ag_out[m * 128:(m + 1) * 128])
            for lc in range(LC):
                nc.sync.dma_start(
                    lat_T[lc], ag_out[(QK + lc) * 128:(QK + lc + 1) * 128])

            # latent transpose -> [k, l] tiles
            with tc.tile_pool(name="psC", bufs=1, space="PSUM") as psC:
                for k in range(SK):
                    for lc in range(LC):
                        tp = psC.tile([128, 128], BF16, name="tp", tag="tp", bufs=4)
                        nc.tensor.transpose(
                            tp, lat_T[lc][:, k * 128:(k + 1) * 128], ident)
                        nc.vector.tensor_copy(
                            lat_kl[k][:, lc * 128:(lc + 1) * 128], tp)

        if upto < 2:
            with tc.tile_pool(name="anchor", bufs=1) as ap_:
                ob = ap_.tile([128, 512], F32, name="ob_anchor")
                nc.vector.tensor_copy(ob[:, 0:KV_LORA], lat_kl[0])
                nc.vector.tensor_copy(ob[:, 0:128], kv_pe[:, 0:128])
                nc.vector.tensor_copy(ob[:, 128:256], a_sb[QK - 1][:, 0:128])
                nc.sync.dma_start(out_d[0:128, 0:512], ob)
            continue

        # ====== phase 2+3: per head pair: q_b, rope, attention ======
        with tc.tile_pool(name="ov", bufs=1) as ovp:
            out_v = [ovp.tile([128, S], BF16, name=f"ov{h}") for h in range(HLOC)]

            with tc.tile_pool(name="ph", bufs=4) as ph, \
                 tc.tile_pool(name="pp", bufs=8) as pp, \
                 tc.tile_pool(name="psH", bufs=1, space="PSUM") as psH:
                for pr in range(NPAIR):
                    # --- q_b for this pair: 4 column chunks of 128 ---
                    q_nope = [ph.tile([128, S], BF16, name=f"qn{e}",
                                      tag=f"qn{e}", bufs=2) for e in (0, 1)]
                    q_pe = ph.tile([128, S], BF16, name="qpe", tag="qpe", bufs=2)
                    q_rot = ph.tile([128, S], BF16, name="qrot", tag="qrot", bufs=2)
                    dsts = [q_nope[0], q_nope[1], q_pe, q_rot]
                    for cc in range(4):
                        w_t = ph.tile([128, QK, 128], BF16, name="qb_t",
                                      tag="qb_stream", bufs=3)
                        nc.sync.dma_start(
                            w_t,
                            qb_d[pr * 4 + cc].rearrange("p (k c) -> p k c", c=128))
                        for qc in range(SQ):
                            sl = slice(qc * 512, (qc + 1) * 512)
                            acc = psH.tile([128, 512], F32, name="acc2",
                                           tag="hmm", bufs=2)
                            for k in range(QK):
                                nc.tensor.matmul(acc, w_t[:, k, :],
                                                 a_sb[k][:, sl],
                                                 start=(k == 0),
                                                 stop=(k == QK - 1))
                            nc.vector.tensor_copy(dsts[cc][:, sl], acc)
                    # rope: q_pe = q_pe*cos + q_rot*sin  (both heads at once)
                    nc.vector.tensor_mul(q_pe, q_pe, cos2)
                    nc.vector.tensor_mul(q_rot, q_rot, sin2)
                    nc.vector.tensor_add(q_pe, q_pe, q_rot)

                    kc_sbs, vc_sbs, qabs = [], [], []
                    for e in (0, 1):
                        h = 2 * pr + e
                        kc_sb = ph.tile([128, KV_LORA], BF16, name=f"kc{e}",
                                        tag=f"kc_w{e}")
                        nc.sync.dma_start(kc_sb, kc_d[h])
                        vc_sb = ph.tile([128, LC, VDIM], BF16, name=f"vc{e}",
                                        tag=f"vc_w{e}")
                        nc.sync.dma_start(
                            vc_sb, vc_d[h].rearrange("(lc p) v -> p lc v", p=128))
                        kc_sbs.append(kc_sb)
                        vc_sbs.append(vc_sb)
                        qabs.append([ph.tile([128, S], BF16, name=f"qabs{e}{lc}",
                                             tag=f"qabs{e}{lc}", bufs=2)
                                     for lc in range(LC)])

                    for e in (0, 1):
                        for lc in range(LC):
                            for qc in range(SQ):
                                sl = slice(qc * 512, (qc + 1) * 512)
                                acc = psH.tile([128, 512], F32, name="acckc",
                                               tag="hmm", bufs=2)
                                nc.tensor.matmul(
                                    acc, kc_sbs[e][:, lc * 128:(lc + 1) * 128],
                                    q_nope[e][:, sl], start=True, stop=True)
                                nc.vector.tensor_copy(qabs[e][lc][:, sl], acc)

                    # --- attention at 256-wide query granularity ---
                    for qv in range(QV):
                        sl = slice(qv * 256, (qv + 1) * 256)
                        nkc = 2 * (qv + 1)
                        for e in (0, 1):
                            h = 2 * pr + e
                            pe_b = e * 64
                            ssum = psH.tile([1, 256], F32, name=f"ssum{e}",
                                            tag="ssum", bufs=2)
                            atts = [psH.tile([128, 256], F32,
                                             name=f"att{lc}",
                                             tag=f"att{lc}", bufs=1)
                                    for lc in range(LC)]
                            for kc in range(nkc):
                                ks = slice(kc * 128, (kc + 1) * 128)
                                sc = psH.tile([128, 256], F32, name="sc",
                                              tag="hmm", bufs=2)
                                for lc in range(LC):
                                    nc.tensor.matmul(sc, lat_T[lc][:, ks],
                                                     qabs[e][lc][:, sl],
                                                     start=(lc == 0), stop=False)
                                nc.tensor.matmul(
                                    sc, kv_pe[pe_b:pe_b + 64, ks],
                                    q_pe[pe_b:pe_b + 64, sl],
                                    start=False, stop=True)
                                probs = pp.tile([128, 256], BF16,
                                                name="probs",
                                                tag="probs", bufs=10)
                                nc.scalar.activation(
                                    probs, sc, mybir.ActivationFunctionType.Exp,
                                    scale=SCALE)
                                midx = kc - 2 * qv
                                if midx >= 0:
                                    nc.vector.tensor_mul(probs, probs,
                                                         masks[midx])
                                nc.tensor.matmul(ssum, ones_col, probs,
                                                 start=(kc == 0),
                                                 stop=(kc == nkc - 1))
                                for lc in range(LC):
                                    nc.tensor.matmul(
                                        atts[lc],
                                        lat_kl[kc][:, lc * 128:(lc + 1) * 128],
                                        probs, start=(kc == 0),
                                        stop=(kc == nkc - 1))
                            recip = ph.tile([1, 256], F32, name=f"recip{e}",
                                            tag="recip", bufs=3)
                            nc.vector.reciprocal(recip, ssum)
                            bc_sb = pp.tile([128, 256], F32, name="bc_sb",
                                            tag="bcsb", bufs=3)
                            nc.gpsimd.partition_broadcast(bc_sb, recip)
                            asb = []
                            for lc in range(LC):
                                t = pp.tile([128, 256], BF16, name=f"asb{lc}",
                                            tag=f"asb{lc}", bufs=2)
                                nc.vector.tensor_copy(t, atts[lc])
                                asb.append(t)
                            vout = psH.tile([128, 256], F32, name="vout",
                                            tag="hmm", bufs=2)
                            for lc in range(LC):
                                nc.tensor.matmul(vout, vc_sbs[e][:, lc, :],
                                                 asb[lc],
                                                 start=(lc == 0),
                                                 stop=(lc == LC - 1))
                            nc.vector.tensor_mul(out_v[h][:, sl], vout, bc_sb)

            # =========== phase 6: o_proj partials ===========
            with tc.tile_pool(name="p6", bufs=6) as p6, \
                 tc.tile_pool(name="ps6", bufs=1, space="PSUM") as ps6:
                for dc in range(D // 512):
                    po = [ps6.tile([128, 512], F32, name=f"po{sc2}",
                                   tag=f"po{sc2}", bufs=1)
                          for sc2 in range(SK)]
                    for hv in range(HLOC):
                        ow_t = p6.tile([128, 512], BF16, name="ow_t",
                                       tag="ow_stream")
                        nc.sync.dma_start(
                            ow_t,
                            ow_d[hv * 128:(hv + 1) * 128, dc * 512:(dc + 1) * 512])
                        for sc2 in range(SK):
                            nc.tensor.matmul(
                                po[sc2], out_v[hv][:, sc2 * 128:(sc2 + 1) * 128],
                                ow_t, start=(hv == 0), stop=(hv == HLOC - 1))
                    for sc2 in range(SK):
                        ob = p6.tile([128, 512], F32, name="ob", tag="ob")
                        nc.scalar.copy(ob, po[sc2])
                        nc.sync.dma_start(
                            out_d[sc2 * 128:(sc2 + 1) * 128,
                                  dc * 512:(dc + 1) * 512], ob)

    nc.compile()
    return nc


def _chunkify(w_chunk):
    """[D, 128] weight column block -> [128, D] in k-major stream layout."""
    return np.ascontiguousarray(
        w_chunk.reshape(DK, 128, 128).transpose(1, 0, 2).reshape(128, D))


def host_prep(hidden_states, cos, sin, q_a_w, q_a_ln_w, q_b_w, kv_a_w,
              kv_a_ln_w, kc_w, vc_w, o_w):
    """Shard + preprocess full fp32 inputs into per-core bf16 in_maps."""
    h = np.asarray(hidden_states)[0]            # [S, D]
    hT = np.ascontiguousarray(h.T).astype(bf16)  # [D, S]
    cosT = np.ascontiguousarray(np.asarray(cos).T)  # [64, S]
    sinT = np.ascontiguousarray(np.asarray(sin).T)
    cos2T = np.concatenate([cosT, cosT], 0).astype(bf16)   # [128, S]
    sin2T = np.concatenate([sinT, sinT], 0).astype(bf16)

    qa0 = np.asarray(q_a_w)                      # [D, 1536]
    lat_w = np.asarray(kv_a_w)[:, :KV_LORA]      # [D, 512]
    pe_w = np.asarray(kv_a_w)[:, KV_LORA:]       # [D, 64]
    rot_w = np.concatenate([-pe_w[:, ROPE // 2:], pe_w[:, :ROPE // 2]], 1)

    # 16 global chunks: 12 aT + 4 latent
    stack = np.concatenate([qa0, lat_w], axis=1)   # [D, 2048]
    pe2 = np.concatenate([pe_w, pe_w], 1)          # [D, 128]
    rot2 = np.concatenate([rot_w, rot_w], 1)
    perot = np.stack([_chunkify(pe2), _chunkify(rot2)]).astype(bf16)

    qb = (np.asarray(q_b_w) * np.asarray(q_a_ln_w)[:, None])  # ln folded

    kv_ln = np.asarray(kv_a_ln_w)
    kc_eff = (np.asarray(kc_w) * kv_ln[None, None, :]).astype(bf16)  # [H,128,512]
    vc_eff = (np.asarray(vc_w) * kv_ln[None, :, None]).astype(bf16)  # [H,512,128]
    ow = np.asarray(o_w)

    i = np.arange(128)[:, None]
    j = np.arange(256)[None, :]
    masks = np.stack([(j >= i + off).astype(np.float32)
                      for off in (0, 128)]).astype(bf16)

    in_maps = []
    for c in range(NCORES):
        # phase-1 chunk shard: global chunks {2c, 2c+1}
        ph1 = np.stack([
            _chunkify(stack[:, (CPC * c + m) * 128:(CPC * c + m + 1) * 128])
            for m in range(CPC)]).astype(bf16)
        is_kv = 1 if CPC * c >= QK else 0
        sel2 = np.zeros((128, 2), np.float32)
        sel2[:, is_kv] = 1.0 / (KV_LORA if is_kv else Q_LORA)
        selb = np.zeros((2, 128), np.float32)
        selb[is_kv, :] = 1.0

        blocks = []
        for p in range(NPAIR):
            h0 = c * HLOC + 2 * p
            h1 = h0 + 1
            n0 = qb[:, h0 * Q_HEAD: h0 * Q_HEAD + NOPE]
            n1 = qb[:, h1 * Q_HEAD: h1 * Q_HEAD + NOPE]
            p0 = qb[:, h0 * Q_HEAD + NOPE: (h0 + 1) * Q_HEAD]
            p1 = qb[:, h1 * Q_HEAD + NOPE: (h1 + 1) * Q_HEAD]
            r0 = np.concatenate([-p0[:, ROPE // 2:], p0[:, :ROPE // 2]], 1)
            r1 = np.concatenate([-p1[:, ROPE // 2:], p1[:, :ROPE // 2]], 1)
            blocks += [n0, n1, np.concatenate([p0, p1], 1),
                       np.concatenate([r0, r1], 1)]
        qb_core0 = np.concatenate(blocks, axis=1).astype(bf16)  # [1536, 4096]
        nqb = QB_COLS // 128
        qb_core = np.ascontiguousarray(
            qb_core0.reshape(QK, 128, nqb, 128).transpose(2, 1, 0, 3)
            .reshape(nqb, 128, Q_LORA))
        in_maps.append({
            "hT": hT,
            "cos2T": cos2T,
            "sin2T": sin2T,
            "ph1_w": ph1,
            "perot_w": perot,
            "sel2": sel2.astype(bf16),
            "selb": selb,
            "qb_w": qb_core,
            "kc_w": kc_eff[c * HLOC:(c + 1) * HLOC],
            "vc_w": vc_eff[c * HLOC:(c + 1) * HLOC],
            "o_w": ow[c * OW_ROWS:(c + 1) * OW_ROWS].astype(bf16),
            "masks": masks,
        })
    return in_maps


_CACHE = {}


def _get_program():
    if "nc" not in _CACHE:
        _CACHE["nc"] = build_program()
    return _CACHE["nc"]


def kernel(**inputs) -> np.ndarray:
    from concourse.bass_utils import run_bass_kernel_spmd
    nc = _get_program()
    in_maps = host_prep(**inputs)
    res = run_bass_kernel_spmd(nc, in_maps, list(range(NCORES)))
    out = np.zeros((S, D), np.float64)
    for c in range(NCORES):
        out += res.results[c]["out"].astype(np.float64)
    return out.astype(np.float32)[None]


# revision 13
# speedup vs baseline: 1.1145x; 1.0205x over previous
"""DeepseekV2 MLA attention kernel for Trainium2, 8-core tensor-parallel.

Strategy: heads sharded across 8 cores (16 each) for attention + o_proj;
the shared q_a / kv_a projections are sharded by output chunk across
cores (2 chunks of 128 each) with an AllReduce for the RMS sums and an
AllGather for the normalized activations (vs. fully replicated in the
baseline -- saves ~250us of tensor-engine time per core).

All device matmuls contract over the SBUF partition dimension
("transposed" [feature, seq] layout):
  phase 1 (sharded): each core computes 2 of 16 global chunks of
    [aT(12); latent(4)] = W_chunk^T @ hT, plus the pe/pe and rot/rot
    chunks replicated (needed for k_pe RoPE with no partition moves).
    Sum-of-squares rows are routed by a per-core selector matmul
    (sel2: [128,2], col r = 1/N for the core's row type), AllReduced,
    rstd broadcast back to 128 partitions via a per-core [2,128]
    selector matmul, chunks normalized locally, then AllGathered.
  phase 2: qT = qb^T @ aT per head pair with pre-rotated pe columns so
    RoPE is elementwise; kc-absorbed q (qabs) per head.
  phase 3: causal attention at 256-wide query granularity (25% less
    masked-tile waste than 512): per (pair, head, 256-q-chunk), scores
    accumulate 5 matmuls per 128-key tile, exp on ACT, diagonal masks,
    softmax denominator via ones-column matmul, single-pass attn
    accumulation (4 latent chunks) in PSUM, vc bmm, recip broadcast.
  phase 6: o_proj accumulated over the 2048 local head*v dims into
    [S, D] partials; host sums the 8 partials.
"""
import sys
import math

sys.path.insert(0, '/opt/trn_rl_repo')

import numpy as np
import ml_dtypes
from contextlib import ExitStack

import concourse.bass as bass
import concourse.tile as tile
from concourse import bacc, mybir
from concourse.masks import make_identity

# ---- problem constants (hardcoded; kernel.py must be self-contained) ----
H = 128
D = 5120
Q_LORA = 1536
KV_LORA = 512
ROPE = 64
NOPE = 128
VDIM = 128
Q_HEAD = NOPE + ROPE
S = 1024
EPS = 1e-6
_MSCALE = 0.1 * 1.0 * math.log(40.0) + 1.0
SCALE = (Q_HEAD ** -0.5) * _MSCALE * _MSCALE

NCORES = 8
HLOC = H // NCORES          # 16 heads per core
QB_COLS = HLOC * (NOPE + 2 * ROPE)   # 4096
OW_ROWS = HLOC * VDIM                # 2048

F32 = mybir.dt.float32
BF16 = mybir.dt.bfloat16

DK = D // 128          # 40 d-chunks
QK = Q_LORA // 128     # 12 q_lora chunks
SQ = S // 512          # 2 free-dim chunks of 512
QV = S // 256          # 4 query chunks of 256 (attention granularity)
SK = S // 128          # 8 key chunks of 128
LC = KV_LORA // 128    # 4 latent chunks
NPAIR = HLOC // 2      # 8 head pairs
NCH = QK + LC          # 16 allgathered chunks
CPC = NCH // NCORES    # 2 chunks per core

bf16 = ml_dtypes.bfloat16


def build_program(reps=1, upto=3):
    nc = bacc.Bacc("TRN2", target_bir_lowering=False, debug=False,
                   num_devices=NCORES)

    hT_d = nc.dram_tensor("hT", [D, S], BF16, kind="ExternalInput").ap()
    cos2_d = nc.dram_tensor("cos2T", [128, S], BF16, kind="ExternalInput").ap()
    sin2_d = nc.dram_tensor("sin2T", [128, S], BF16, kind="ExternalInput").ap()
    ph1_d = nc.dram_tensor("ph1_w", [CPC, DK, 128, 128], BF16,
                           kind="ExternalInput").ap()
    perot_d = nc.dram_tensor("perot_w", [2, DK, 128, 128], BF16,
                             kind="ExternalInput").ap()
    sel2_d = nc.dram_tensor("sel2", [128, 2], BF16, kind="ExternalInput").ap()
    selq_d = nc.dram_tensor("selq", [2, 128], F32, kind="ExternalInput").ap()
    selkv_d = nc.dram_tensor("selkv", [2, 128], F32, kind="ExternalInput").ap()
    qb_d = nc.dram_tensor("qb_w", [QB_COLS // 128, 128, Q_LORA], BF16,
                          kind="ExternalInput").ap()
    kc_d = nc.dram_tensor("kc_w", [HLOC, NOPE, KV_LORA], BF16, kind="ExternalInput").ap()
    vc_d = nc.dram_tensor("vc_w", [HLOC, KV_LORA, VDIM], BF16, kind="ExternalInput").ap()
    ow_d = nc.dram_tensor("o_w", [OW_ROWS, D], BF16, kind="ExternalInput").ap()
    mask_d = nc.dram_tensor("masks", [2, 128, 256], BF16, kind="ExternalInput").ap()
    out_d = nc.dram_tensor("out", [S, D], F32, kind="ExternalOutput").ap()
    rg = [list(range(NCORES))]

    with tile.TileContext(nc) as tc, \
         nc.allow_low_precision(reason="float32r rows are fp32-width"):
      for _rep in range(reps):
       with ExitStack() as ctx:
        const = ctx.enter_context(tc.tile_pool(name="const", bufs=1))
        persist = ctx.enter_context(tc.tile_pool(name="persist", bufs=1))
        dram = ctx.enter_context(tc.tile_pool(name="dram", bufs=1, space="DRAM"))

        # ---- constants ----
        ident = const.tile([128, 128], BF16)
        make_identity(nc, ident)
        ones_col = const.tile([128, 1], BF16)   # lhsT for partition sums
        nc.vector.memset(ones_col, 1.0)
        eps_sb2 = const.tile([2, 1], F32)
        nc.vector.memset(eps_sb2, EPS)
        sel2 = const.tile([128, 2], BF16)
        nc.sync.dma_start(sel2, sel2_d)
        ones_row = const.tile([1, 128], F32)   # lhsT for [1,N]->[128,N] bcast
        nc.vector.memset(ones_row, 1.0)
        selq = const.tile([2, 128], F32)       # lhsT picking rstd row 0
        nc.sync.dma_start(selq, selq_d)
        selkv = const.tile([2, 128], F32)      # lhsT picking rstd row 1
        nc.sync.dma_start(selkv, selkv_d)
        cos2 = const.tile([128, S], BF16)
        nc.sync.dma_start(cos2, cos2_d)
        sin2 = const.tile([128, S], BF16)
        nc.sync.dma_start(sin2, sin2_d)
        masks = []
        for i in range(2):
            m_t = const.tile([128, 256], BF16, name=f"mask{i}")
            nc.sync.dma_start(m_t, mask_d[i])
            masks.append(m_t)

        # persistent activations (whole-program scope)
        a_sb = [persist.tile([128, S], BF16, name=f"a{m}") for m in range(QK)]
        lat_T = [persist.tile([128, S], BF16, name=f"latT{m}") for m in range(LC)]
        lat_kl = [persist.tile([128, KV_LORA], BF16, name=f"latkl{k}")
                  for k in range(SK)]
        kv_pe = persist.tile([128, S], BF16, name="kv_pe")

        # collective bounce buffers
        ar_in = dram.tile([2, S], F32)
        ar_out = dram.tile([2, S], F32, addr_space="Shared")
        ag_in = dram.tile([CPC * 128, S], BF16)
        ag_out = dram.tile([NCH * 128, S], BF16, addr_space="Shared")

        # =========== phase 1 (sharded): own chunks + replicated pe/rot ====
        # k-outer single pass: all 4 chunk accumulators (2 own + 2 perot)
        # advance together per hT k-chunk, so hT streams through a small
        # rotating window (1.5MB instead of 10.5MB resident) and each
        # stationary weight is loaded once per (k, chunk).
        with tc.tile_pool(name="p1", bufs=2) as p1, \
             tc.tile_pool(name="p1s", bufs=1) as p1s:
            w_ts = [p1.tile([128, DK, 128], BF16, name=f"wt{m}", bufs=1)
                    for m in range(CPC + 2)]
            own = [p1s.tile([128, S], BF16, name=f"own{m}") for m in range(CPC)]
            kv_pe_raw = p1s.tile([128, S], BF16, name="kv_pe_raw")
            kv_rot = p1s.tile([128, S], BF16, name="kv_rot")
            ss_sb = p1s.tile([2, S], F32, name="ss_sb")
            dsts = list(own) + [kv_pe_raw, kv_rot]

            with tc.tile_pool(name="psA", bufs=1, space="PSUM") as psA:
                accs = [[psA.tile([128, 512], F32, name=f"acc{m}{qc}",
                                  tag=f"acc{m}{qc}", bufs=1)
                         for qc in range(SQ)] for m in range(CPC + 2)]
                for k in range(DK):
                    h_t = p1.tile([128, S], BF16, name="h_t", tag="hT", bufs=6)
                    nc.sync.dma_start(h_t, hT_d[k * 128:(k + 1) * 128, :])
                    for m in range(CPC):
                        nc.sync.dma_start(w_ts[m][:, k, :], ph1_d[m, k])
                    for m in range(2):
                        nc.sync.dma_start(w_ts[CPC + m][:, k, :], perot_d[m, k])
                    for m in range(CPC + 2):
                        for qc in range(SQ):
                            sl = slice(qc * 512, (qc + 1) * 512)
                            nc.tensor.matmul(accs[m][qc], w_ts[m][:, k, :],
                                             h_t[:, sl],
                                             start=(k == 0), stop=(k == DK - 1))
                for m in range(CPC + 2):
                    for qc in range(SQ):
                        sl = slice(qc * 512, (qc + 1) * 512)
                        nc.vector.tensor_copy(dsts[m][:, sl], accs[m][qc])

                # sum-of-squares rows (from SBUF copies), routed by sel2
                ss2 = [psA.tile([2, 512], F32, name=f"ss2_{qc}",
                                tag="acc00", bufs=1) for qc in range(SQ)]
                for qc in range(SQ):
                    sl = slice(qc * 512, (qc + 1) * 512)
                    for m in range(CPC):
                        sq = p1.tile([128, 512], BF16, name="sq", tag="sq")
                        nc.scalar.square(sq, own[m][:, sl])
                        nc.tensor.matmul(ss2[qc], sel2, sq,
                                         start=(m == 0), stop=(m == CPC - 1))
                for qc in range(SQ):
                    sl = slice(qc * 512, (qc + 1) * 512)
                    nc.vector.tensor_copy(ss_sb[:, sl], ss2[qc])
                # AR (sums) and AG (raw chunks) fly concurrently; rstd is
                # applied post-gather
                nc.sync.dma_start(ar_in[:], ss_sb)
                nc.gpsimd.collective_compute(
                    "AllReduce", mybir.AluOpType.add, replica_groups=rg,
                    ins=[ar_in[:]], outs=[ar_out[:]])
                for m in range(CPC):
                    nc.sync.dma_start(ag_in[m * 128:(m + 1) * 128], own[m])
                nc.gpsimd.collective_compute(
                    "AllGather", mybir.AluOpType.bypass, replica_groups=rg,
                    ins=[ag_in[:]], outs=[ag_out[:]])

                # k_pe rope (pe duplicated in both partition halves)
                nc.vector.tensor_mul(kv_pe, kv_pe_raw, cos2)
                nc.vector.tensor_mul(kv_rot, kv_rot, sin2)
                nc.vector.tensor_add(kv_pe, kv_pe, kv_rot)

                # rstd rows from the allreduced sums
                rstd_src = p1s.tile([2, S], F32, name="rstd_src")
                nc.sync.dma_start(rstd_src, ar_out[:])
                rstd2 = p1s.tile([2, S], F32, name="rstd2")
                nc.scalar.activation(rstd2, rstd_src,
                                     mybir.ActivationFunctionType.Sqrt,
                                     bias=eps_sb2, scale=1.0)
                nc.vector.reciprocal(rstd2, rstd2)

                # read back gathered raw activations, normalize in place
                for m in range(QK):
                    nc.sync.dma_start(a_sb[m], ag_out[m * 128:(m + 1) * 128])
                for lc in range(LC):
                    nc.sync.dma_start(
                        lat_T[lc], ag_out[(QK + lc) * 128:(QK + lc + 1) * 128])
                for qc in range(SQ):
                    sl = slice(qc * 512, (qc + 1) * 512)
                    bc_q = psA.tile([128, 512], F32, name="bc_q",
                                    tag="acc10", bufs=1)
                    nc.tensor.matmul(bc_q, selq, rstd2[:, sl],
                                     start=True, stop=True)
                    for m in range(QK):
                        nc.vector.tensor_mul(a_sb[m][:, sl], a_sb[m][:, sl],
                                             bc_q)
                    bc_kv = psA.tile([128, 512], F32, name="bc_kv",
                                     tag="acc20", bufs=1)
                    nc.tensor.matmul(bc_kv, selkv, rstd2[:, sl],
                                     start=True, stop=True)
                    for lc in range(LC):
                        nc.vector.tensor_mul(lat_T[lc][:, sl], lat_T[lc][:, sl],
                                             bc_kv)

            # latent transpose -> [k, l] tiles
            with tc.tile_pool(name="psC", bufs=1, space="PSUM") as psC:
                for k in range(SK):
                    for lc in range(LC):
                        tp = psC.tile([128, 128], BF16, name="tp", tag="tp", bufs=4)
                        nc.tensor.transpose(
                            tp, lat_T[lc][:, k * 128:(k + 1) * 128], ident)
                        nc.vector.tensor_copy(
                            lat_kl[k][:, lc * 128:(lc + 1) * 128], tp)

        if upto < 2:
            with tc.tile_pool(name="anchor", bufs=1) as ap_:
                ob = ap_.tile([128, 512], F32, name="ob_anchor")
                nc.vector.tensor_copy(ob[:, 0:KV_LORA], lat_kl[0])
                nc.vector.tensor_copy(ob[:, 0:128], kv_pe[:, 0:128])
                nc.vector.tensor_copy(ob[:, 128:256], a_sb[QK - 1][:, 0:128])
                nc.sync.dma_start(out_d[0:128, 0:512], ob)
            continue

        # ====== phase 2+3: per head pair: q_b, rope, attention ======
        with tc.tile_pool(name="ov", bufs=1) as ovp:
            out_v = [ovp.tile([128, S], BF16, name=f"ov{h}") for h in range(HLOC)]

            with tc.tile_pool(name="ph", bufs=4) as ph, \
                 tc.tile_pool(name="pp", bufs=8) as pp, \
                 tc.tile_pool(name="psH", bufs=1, space="PSUM") as psH:
                for pr in range(NPAIR):
                    # --- q_b for this pair: 4 column chunks of 128 ---
                    q_nope = [ph.tile([128, S], BF16, name=f"qn{e}",
                                      tag=f"qn{e}", bufs=2) for e in (0, 1)]
                    q_pe = ph.tile([128, S], BF16, name="qpe", tag="qpe", bufs=2)
                    q_rot = ph.tile([128, S], BF16, name="qrot", tag="qrot", bufs=2)
                    dsts = [q_nope[0], q_nope[1], q_pe, q_rot]
                    for cc in range(4):
                        w_t = ph.tile([128, QK, 128], BF16, name="qb_t",
                                      tag="qb_stream", bufs=4)
                        nc.sync.dma_start(
                            w_t,
                            qb_d[pr * 4 + cc].rearrange("p (k c) -> p k c", c=128))
                        for qc in range(SQ):
                            sl = slice(qc * 512, (qc + 1) * 512)
                            acc = psH.tile([128, 512], F32, name="acc2",
                                           tag="hmm", bufs=3)
                            for k in range(QK):
                                nc.tensor.matmul(acc, w_t[:, k, :],
                                                 a_sb[k][:, sl],
                                                 start=(k == 0),
                                                 stop=(k == QK - 1))
                            nc.vector.tensor_copy(dsts[cc][:, sl], acc)
                    # rope: q_pe = q_pe*cos + q_rot*sin  (both heads at once)
                    nc.vector.tensor_mul(q_pe, q_pe, cos2)
                    nc.vector.tensor_mul(q_rot, q_rot, sin2)
                    nc.vector.tensor_add(q_pe, q_pe, q_rot)

                    kc_sbs, vc_sbs, qabs = [], [], []
                    for e in (0, 1):
                        h = 2 * pr + e
                        kc_sb = ph.tile([128, KV_LORA], BF16, name=f"kc{e}",
                                        tag=f"kc_w{e}")
                        nc.sync.dma_start(kc_sb, kc_d[h])
                        vc_sb = ph.tile([128, LC, VDIM], BF16, name=f"vc{e}",
                                        tag=f"vc_w{e}")
                        nc.sync.dma_start(
                            vc_sb, vc_d[h].rearrange("(lc p) v -> p lc v", p=128))
                        kc_sbs.append(kc_sb)
                        vc_sbs.append(vc_sb)
                        qabs.append([ph.tile([128, S], BF16, name=f"qabs{e}{lc}",
                                             tag=f"qabs{e}{lc}", bufs=2)
                                     for lc in range(LC)])

                    for e in (0, 1):
                        for lc in range(LC):
                            for qc in range(SQ):
                                sl = slice(qc * 512, (qc + 1) * 512)
                                acc = psH.tile([128, 512], F32, name="acckc",
                                               tag="hmm", bufs=3)
                                nc.tensor.matmul(
                                    acc, kc_sbs[e][:, lc * 128:(lc + 1) * 128],
                                    q_nope[e][:, sl], start=True, stop=True)
                                nc.vector.tensor_copy(qabs[e][lc][:, sl], acc)

                    # --- attention at 256-wide query granularity ---
                    for qv in range(QV):
                        sl = slice(qv * 256, (qv + 1) * 256)
                        nkc = 2 * (qv + 1)
                        for e in (0, 1):
                            h = 2 * pr + e
                            pe_b = e * 64
                            ssum = psH.tile([1, 256], F32, name=f"ssum{e}",
                                            tag="ssum", bufs=1)
                            atts = [psH.tile([128, 256], F32,
                                             name=f"att{lc}",
                                             tag=f"att{lc}", bufs=1)
                                    for lc in range(LC)]
                            for kc in range(nkc):
                                ks = slice(kc * 128, (kc + 1) * 128)
                                sc = psH.tile([128, 256], F32, name="sc",
                                              tag="hmm", bufs=3)
                                for lc in range(LC):
                                    nc.tensor.matmul(sc, lat_T[lc][:, ks],
                                                     qabs[e][lc][:, sl],
                                                     start=(lc == 0), stop=False)
                                nc.tensor.matmul(
                                    sc, kv_pe[pe_b:pe_b + 64, ks],
                                    q_pe[pe_b:pe_b + 64, sl],
                                    start=False, stop=True)
                                probs = pp.tile([128, 256], BF16,
                                                name="probs",
                                                tag="probs", bufs=10)
                                nc.scalar.activation(
                                    probs, sc, mybir.ActivationFunctionType.Exp,
                                    scale=SCALE)
                                midx = kc - 2 * qv
                                if midx >= 0:
                                    nc.vector.tensor_mul(probs, probs,
                                                         masks[midx])
                                nc.tensor.matmul(ssum, ones_col, probs,
                                                 start=(kc == 0),
                                                 stop=(kc == nkc - 1))
                                for lc in range(LC):
                                    nc.tensor.matmul(
                                        atts[lc],
                                        lat_kl[kc][:, lc * 128:(lc + 1) * 128],
                                        probs, start=(kc == 0),
                                        stop=(kc == nkc - 1))
                            recip = ph.tile([1, 256], F32, name=f"recip{e}",
                                            tag="recip", bufs=3)
                            nc.vector.reciprocal(recip, ssum)
                            bc_sb = pp.tile([128, 256], F32, name="bc_sb",
                                            tag="bcsb", bufs=3)
                            nc.gpsimd.partition_broadcast(bc_sb, recip)
                            asb = []
                            for lc in range(LC):
                                t = pp.tile([128, 256], BF16, name=f"asb{lc}",
                                            tag=f"asb{lc}", bufs=2)
                                nc.vector.tensor_copy(t, atts[lc])
                                asb.append(t)
                            vout = psH.tile([128, 256], F32, name="vout",
                                            tag="hmm", bufs=3)
                            for lc in range(LC):
                                nc.tensor.matmul(vout, vc_sbs[e][:, lc, :],
                                                 asb[lc],
                                                 start=(lc == 0),
                                                 stop=(lc == LC - 1))
                            nc.vector.tensor_mul(out_v[h][:, sl], vout, bc_sb)

            # =========== phase 6: o_proj partials ===========
            with tc.tile_pool(name="p6", bufs=6) as p6, \
                 tc.tile_pool(name="ps6", bufs=1, space="PSUM") as ps6:
                for dc in range(D // 512):
                    po = [ps6.tile([128, 512], F32, name=f"po{sc2}",
                                   tag=f"po{sc2}", bufs=1)
                          for sc2 in range(SK)]
                    for hv in range(HLOC):
                        ow_t = p6.tile([128, 512], BF16, name="ow_t",
                                       tag="ow_stream")
                        nc.sync.dma_start(
                            ow_t,
                            ow_d[hv * 128:(hv + 1) * 128, dc * 512:(dc + 1) * 512])
                        for sc2 in range(SK):
                            nc.tensor.matmul(
                                po[sc2], out_v[hv][:, sc2 * 128:(sc2 + 1) * 128],
                                ow_t, start=(hv == 0), stop=(hv == HLOC - 1))
                    for sc2 in range(SK):
                        ob = p6.tile([128, 512], F32, name="ob", tag="ob")
                        nc.scalar.copy(ob, po[sc2])
                        nc.sync.dma_start(
                            out_d[sc2 * 128:(sc2 + 1) * 128,
                                  dc * 512:(dc + 1) * 512], ob)

    nc.compile()
    return nc


def _chunkify(w_chunk):
    """[D, 128] weight column block -> [DK, 128, 128] contiguous k-blocks."""
    return np.ascontiguousarray(w_chunk.reshape(DK, 128, 128))


def host_prep(hidden_states, cos, sin, q_a_w, q_a_ln_w, q_b_w, kv_a_w,
              kv_a_ln_w, kc_w, vc_w, o_w):
    """Shard + preprocess full fp32 inputs into per-core bf16 in_maps."""
    h = np.asarray(hidden_states)[0]            # [S, D]
    hT = np.ascontiguousarray(h.T).astype(bf16)  # [D, S]
    cosT = np.ascontiguousarray(np.asarray(cos).T)  # [64, S]
    sinT = np.ascontiguousarray(np.asarray(sin).T)
    cos2T = np.concatenate([cosT, cosT], 0).astype(bf16)   # [128, S]
    sin2T = np.concatenate([sinT, sinT], 0).astype(bf16)

    qa0 = np.asarray(q_a_w)                      # [D, 1536]
    lat_w = np.asarray(kv_a_w)[:, :KV_LORA]      # [D, 512]
    pe_w = np.asarray(kv_a_w)[:, KV_LORA:]       # [D, 64]
    rot_w = np.concatenate([-pe_w[:, ROPE // 2:], pe_w[:, :ROPE // 2]], 1)

    # 16 global chunks: 12 aT + 4 latent
    stack = np.concatenate([qa0, lat_w], axis=1)   # [D, 2048]
    pe2 = np.concatenate([pe_w, pe_w], 1)          # [D, 128]
    rot2 = np.concatenate([rot_w, rot_w], 1)
    perot = np.stack([_chunkify(pe2), _chunkify(rot2)]).astype(bf16)

    qb = (np.asarray(q_b_w) * np.asarray(q_a_ln_w)[:, None])  # ln folded

    kv_ln = np.asarray(kv_a_ln_w)
    kc_eff = (np.asarray(kc_w) * kv_ln[None, None, :]).astype(bf16)  # [H,128,512]
    vc_eff = (np.asarray(vc_w) * kv_ln[None, :, None]).astype(bf16)  # [H,512,128]
    ow = np.asarray(o_w)

    i = np.arange(128)[:, None]
    j = np.arange(256)[None, :]
    masks = np.stack([(j >= i + off).astype(np.float32)
                      for off in (0, 128)]).astype(bf16)

    in_maps = []
    for c in range(NCORES):
        # phase-1 chunk shard: global chunks {2c, 2c+1}
        ph1 = np.stack([
            _chunkify(stack[:, (CPC * c + m) * 128:(CPC * c + m + 1) * 128])
            for m in range(CPC)]).astype(bf16)
        is_kv = 1 if CPC * c >= QK else 0
        sel2 = np.zeros((128, 2), np.float32)
        sel2[:, is_kv] = 1.0 / (KV_LORA if is_kv else Q_LORA)
        selb = np.zeros((2, 128), np.float32)
        selb[is_kv, :] = 1.0

        blocks = []
        for p in range(NPAIR):
            h0 = c * HLOC + 2 * p
            h1 = h0 + 1
            n0 = qb[:, h0 * Q_HEAD: h0 * Q_HEAD + NOPE]
            n1 = qb[:, h1 * Q_HEAD: h1 * Q_HEAD + NOPE]
            p0 = qb[:, h0 * Q_HEAD + NOPE: (h0 + 1) * Q_HEAD]
            p1 = qb[:, h1 * Q_HEAD + NOPE: (h1 + 1) * Q_HEAD]
            r0 = np.concatenate([-p0[:, ROPE // 2:], p0[:, :ROPE // 2]], 1)
            r1 = np.concatenate([-p1[:, ROPE // 2:], p1[:, :ROPE // 2]], 1)
            blocks += [n0, n1, np.concatenate([p0, p1], 1),
                       np.concatenate([r0, r1], 1)]
        qb_core0 = np.concatenate(blocks, axis=1).astype(bf16)  # [1536, 4096]
        nqb = QB_COLS // 128
        qb_core = np.ascontiguousarray(
            qb_core0.reshape(QK, 128, nqb, 128).transpose(2, 1, 0, 3)
            .reshape(nqb, 128, Q_LORA))
        in_maps.append({
            "hT": hT,
            "cos2T": cos2T,
            "sin2T": sin2T,
            "ph1_w": ph1,
            "perot_w": perot,
            "sel2": sel2.astype(bf16),
            "selq": np.stack([np.ones(128, np.float32),
                              np.zeros(128, np.float32)]),
            "selkv": np.stack([np.zeros(128, np.float32),
                               np.ones(128, np.float32)]),
            "selb": selb,
            "qb_w": qb_core,
            "kc_w": kc_eff[c * HLOC:(c + 1) * HLOC],
            "vc_w": vc_eff[c * HLOC:(c + 1) * HLOC],
            "o_w": ow[c * OW_ROWS:(c + 1) * OW_ROWS].astype(bf16),
            "masks": masks,
        })
    return in_maps


_CACHE = {}


def _get_program():
    if "nc" not in _CACHE:
        _CACHE["nc"] = build_program()
    return _CACHE["nc"]


def kernel(**inputs) -> np.ndarray:
    from concourse.bass_utils import run_bass_kernel_spmd
    nc = _get_program()
    in_maps = host_prep(**inputs)
    res = run_bass_kernel_spmd(nc, in_maps, list(range(NCORES)))
    out = np.zeros((S, D), np.float64)
    for c in range(NCORES):
        out += res.results[c]["out"].astype(np.float64)
    return out.astype(np.float32)[None]
